# revision 58
# baseline (speedup 1.0000x reference)
import sys
import numpy as np
import ml_dtypes

sys.path.insert(0, '/opt/trn_rl_repo')

import concourse.bacc as bacc
import concourse.mybir as mybir
from concourse.bass_utils import run_bass_kernel_spmd
from concourse.tile import TileContext
from contextlib import ExitStack

f32 = mybir.dt.float32
bf16 = mybir.dt.bfloat16
AF = mybir.ActivationFunctionType
ALU = mybir.AluOpType

D_MODEL = 1024
N_HEAD = 16
D_HEAD = 64
B = 4
T = 2048
N_CORES = 8
HPC = N_HEAD // 2        # 8 heads per core
HD = HPC * D_HEAD        # 512 head-dims per core
NTK = D_MODEL // 128     # 8 k-chunks over model dim
NTT = T // 128           # 16 T-tiles of 128
JC_ORDER = [0, 1, 2, 3]
N_WARM = 64              # PE warm-up matmuls during the input-DMA window

_cache = {}


def _build():
    nc = bacc.Bacc()
    # host-side layouts: [128, k, cols] so each logical tensor loads in one DMA
    xT = nc.declare_dram_parameter("xT", [128, NTK, T], bf16, isOutput=False)
    wqkT = nc.declare_dram_parameter("wqkT", [128, NTK, 2 * HD], bf16,
                                     isOutput=False)
    wvT = nc.declare_dram_parameter("wvT", [128, NTK, HD], bf16, isOutput=False)
    wpT = nc.declare_dram_parameter("wpT", [128, HD // 128, D_MODEL], bf16,
                                    isOutput=False)
    trimask = nc.declare_dram_parameter("trimask", [128, 128], bf16,
                                        isOutput=False)
    identT = nc.declare_dram_parameter("identT", [128, 128], bf16,
                                       isOutput=False)
    outp = nc.declare_dram_parameter("out", [T, D_MODEL], f32, isOutput=True)

    with TileContext(nc) as tc, ExitStack() as outer:
        # ---- pools ----
        qkp = outer.enter_context(tc.tile_pool(name="qk", bufs=1))
        vp = outer.enter_context(tc.tile_pool(name="v", bufs=1))
        smp = outer.enter_context(tc.tile_pool(name="small", bufs=1))
        ywp = outer.enter_context(tc.tile_pool(name="yw", bufs=1))
        psA = outer.enter_context(tc.tile_pool(name="psA", bufs=2, space="PSUM"))
        psY = outer.enter_context(tc.tile_pool(name="psY", bufs=1, space="PSUM"))
        ptp = outer.enter_context(tc.tile_pool(name="pt", bufs=11))
        rp = outer.enter_context(tc.tile_pool(name="r", bufs=4))
        xp = outer.enter_context(tc.tile_pool(name="x", bufs=1))
        wqp = outer.enter_context(tc.tile_pool(name="wq", bufs=1))
        wvp = outer.enter_context(tc.tile_pool(name="wv", bufs=1))
        ps1 = outer.enter_context(tc.tile_pool(name="ps1", bufs=2, space="PSUM"))
        obp = outer.enter_context(tc.tile_pool(name="ob", bufs=6))

        qk = [qkp.tile([128, T], bf16, tag=f"qk{m}", name=f"qk{m}") for m in range(8)]
        vt = [vp.tile([128, HPC * 65], bf16, tag=f"v{t}", name=f"va{t}")
              for t in range(NTT)]
        mask = smp.tile([128, 128], bf16, name="maskt")
        ident = smp.tile([128, 128], bf16, name="identt")
        ynall = ywp.tile([128, NTT, HD], bf16, name="ynall")     # y norm [q, (j, 8h*64)]
        ysbT = ywp.tile([128, HD // 128, T], bf16, name="ysbT")  # y^T [d, slice, t]
        wp = ywp.tile([128, HD // 128, D_MODEL], bf16, name="wpt")
        xt = xp.tile([128, NTK, T], bf16, name="xtt")
        wq = wqp.tile([128, NTK, 2 * HD], bf16, name="wqt")
        wv = wvp.tile([128, NTK, HD], bf16, name="wvt")

        # ---- input DMAs. DMA rate scales with bytes-per-partition-line, so
        # everything early-critical moves as per-k [128, 512] chunks (1KB
        # lines) spread across SP+ACT; wqkT's host layout interleaves the
        # per-head-pair q/k column blocks so cols 0:512 cover m0+m1 whole.
        # gpsimd (SWDGE, slow start) only carries late bulk. ----
        nc.scalar.dma_start(out=mask[:], in_=trimask[:, :])
        nc.scalar.dma_start(out=xt[:, 0:4, 0:512], in_=xT[:, 0:4, 0:512])
        nc.sync.dma_start(out=wq[:, 0:4, 0:512], in_=wqkT[:, 0:4, 0:512])
        nc.sync.dma_start(out=wq[:, 4:8, 0:512], in_=wqkT[:, 4:8, 0:512])
        nc.sync.dma_start(out=xt[:, 4:8, 0:512], in_=xT[:, 4:8, 0:512])
        nc.sync.dma_start(out=wv[:, 0:4, :], in_=wvT[:, 0:4, :])
        nc.sync.dma_start(out=wv[:, 4:8, :], in_=wvT[:, 4:8, :])
        nc.sync.dma_start(out=wq[:, :, 512:1024], in_=wqkT[:, :, 512:1024])
        nc.sync.dma_start(out=xt[:, :, 512:1024], in_=xT[:, :, 512:1024])
        nc.sync.dma_start(out=xt[:, :, 1024:1536], in_=xT[:, :, 1024:1536])
        nc.sync.dma_start(out=wp[:, :, :], in_=wpT[:, :, :])
        nc.sync.dma_start(out=xt[:, :, 1536:2048], in_=xT[:, :, 1536:2048])

        # warm the ACT exp table early (one-time table-load off critical path)
        warm = smp.tile([1, 16], f32, name="warmt")
        nc.vector.memset(warm[:], 0.0)
        nc.scalar.activation(warm[:], warm[:], AF.Exp)
        # warm the GPSIMD ucode library (~70us lazy load on first compute op),
        # then late bulk on the gpsimd queue
        gwarm = smp.tile([2, 16], bf16, name="gwarmt")
        nc.gpsimd.memset(gwarm[:], 1.0)
        nc.gpsimd.tensor_tensor(gwarm[:], gwarm[:], gwarm[:], ALU.mult)
        nc.gpsimd.dma_start(out=ident[:], in_=identT[:, :])
        # warm the PE/HAM during the DMA window: dummy matmuls on the mask
        wps = ps1.tile([128, 512], f32, tag="ps1", name="warmmm")
        for _ in range(N_WARM):
            nc.tensor.matmul(wps[:, 0:128], mask[:], mask[:],
                             start=True, stop=True)

        # ---- global step list ----
        STEPS = []            # (rnd, jc, m, i)
        g_of = {}             # (rnd, i) -> global index
        rnd = 0
        for jc in JC_ORDER:
            for m in range(4):
                for i in range(4 * jc + 4):
                    g_of[(rnd, i)] = len(STEPS)
                    STEPS.append((rnd, jc, m, i))
                rnd += 1
        NSTEPS = len(STEPS)
        LAST_RND = rnd - 1
        RLEN = [4 * jc + 4 for jc in JC_ORDER for _ in range(4)]

        # ---- work-unit emitters (each unit = 2 parts sharing one psum tile)
        s1a_done = set()
        s1b_done = set()
        ns_pe = [0.0]         # emitted PE stream time (ns)
        ns_act = [0.0]        # emitted ACT stream time (ns)

        def s1a_parts(mq, j):
            # two self-contained 256-col parts (own psum tile + copy each)
            # wqkT columns are interleaved per head-pair: [q0 k0 q1 k1 ...]
            wcol = 256 * mq if mq < 4 else 256 * (mq - 4) + 128

            def part(half):
                c0 = j * 512 + half * 256
                ps = ps1.tile([128, 512], f32, tag="ps1", name="ps1t")
                for k in range(NTK):
                    nc.tensor.matmul(ps[:, 0:256],
                                     wq[:, k, wcol:wcol + 128],
                                     xt[:, k, c0:c0 + 256],
                                     start=(k == 0), stop=(k == NTK - 1))
                nc.vector.tensor_copy(qk[mq][:, c0:c0 + 256], ps[:, 0:256])
                ns_pe[0] += 4 * 512 / 2.4
            return [lambda h=h: part(h) for h in range(2)]

        def s1b_parts(t):
            # two self-contained 4-head parts (own psum tile + copy each)
            def part(half):
                va3 = vt[t][:].rearrange("p (h e) -> p h e", e=65)
                if half == 0:
                    nc.vector.memset(va3[:, :, 64], 1.0)
                ps = ps1.tile([128, 512], f32, tag="ps1", name="ps1vt")
                for k in range(NTK):
                    nc.tensor.matmul(ps[:, 0:256],
                                     xt[:, k, t * 128:(t + 1) * 128],
                                     wv[:, k, half * 256:half * 256 + 256],
                                     start=(k == 0), stop=(k == NTK - 1))
                nc.vector.tensor_copy(
                    va3[:, 4 * half:4 * half + 4, 0:64],
                    ps[:, 0:256].rearrange("p (h e) -> p h e", e=64))
                ns_pe[0] += 4 * 512 / 2.4
            return [lambda h=h: part(h) for h in range(2)]

        def s4_part(j, oc, big=None):
            ps = big[:, 512 * oc:512 * (oc + 1)] if big is not None else \
                ps1.tile([128, 512], f32, tag="ps1", name="ps4t")[:]
            for k in range(HD // 128):
                nc.tensor.matmul(
                    ps, ysbT[:, k, 128 * j:128 * j + 128],
                    wp[:, k, oc * 512:(oc + 1) * 512],
                    start=(k == 0), stop=(k == HD // 128 - 1))
            o_ = obp.tile([128, 512], f32, tag="o", name="obt")
            nc.vector.tensor_copy(o_[:], ps)
            nc.sync.dma_start(
                out=outp[j * 128:(j + 1) * 128, oc * 512:(oc + 1) * 512],
                in_=o_[:])
            ns_pe[0] += 4 * 512 / 2.4

        def s4_tail(j):
            # borrow a psA-pool tile (2 banks) so tail units pipeline
            big = psA.tile([128, 1024], f32, tag="psa", name="ps4big")
            s4_part(j, 0, big=big)
            s4_part(j, 1, big=big)

        # ---- fillers: [deadline_g, parts] kept deadline-sorted ----
        fillers = []
        staged = []           # [activation_g, deadline_g, parts]

        def drain_fillers(g, force_all=False):
            for s in staged[:]:
                if force_all or s[0] <= g:
                    fillers.append(s[1:])
                    staged.remove(s)
            fillers.sort(key=lambda f: f[0])
            # Trickle parts (<=2 per phase) ahead of their deadlines so
            # deadline clusters at jc transitions never dump several units
            # between consecutive S2s (which stalls the exp stream). The
            # hard force at dl<=g is the correctness backstop. The credit
            # gate (PE plan ~1.06x ACT stream) paces far-future fillers.
            emitted = 0
            while fillers:
                dl, parts = fillers[0]
                if force_all or dl <= g:
                    while parts:
                        parts.pop(0)()
                    fillers.pop(0)
                    continue
                limit = 2 if dl <= g + 4 else 1
                if emitted < limit and (
                        dl <= g + 12
                        or ns_pe[0] < ns_act[0] * 1.065 + 300):
                    parts.pop(0)()
                    emitted += 1
                    if not parts:
                        fillers.pop(0)
                    continue
                break

        def sched_s1a(mq, j, deadline):
            if (mq, j) in s1a_done:
                return
            s1a_done.add((mq, j))
            fillers.append([deadline, s1a_parts(mq, j)])

        def sched_s1b(t, deadline):
            if t in s1b_done:
                return
            s1b_done.add(t)
            fillers.append([deadline, s1b_parts(t)])

        # round 0 q/k projections run inline behind the warm-up
        s1a_done.add((0, 0))
        s1a_done.add((4, 0))
        for p in s1a_parts(0, 0):
            p()
        for p in s1a_parts(4, 0):
            p()

        # schedule all other projection units with deadlines
        for t in range(NTT):
            r0 = 4 * (t // 4)                      # first round of jc = t//4
            sched_s1b(t, max(0, g_of[(r0, t)]))
        rnd = 0
        for jc in JC_ORDER:
            for m in range(4):
                sched_s1a(m, jc, max(0, g_of[(rnd, 0)] - 3))
                for jcp in range(jc + 1):
                    sched_s1a(4 + m, jcp, max(0, g_of[(rnd, 4 * jcp)] - 3))
                rnd += 1

        # ---- per-step state ----
        psas = {}
        pts = {}
        psy_cur = {}          # rnd -> (psy3A, psy3B, hA, hB, jc)

        def emit_s2(g):
            _, jc, m, i = STEPS[g]
            qt, kt = qk[m], qk[4 + m]
            qlo = 512 * jc
            su = max(0, 128 * i - qlo)
            psa = psA.tile([128, 1024], f32, tag="psa", name="psat")
            psas[g] = (psa, su)
            nc.tensor.matmul(
                psa[:, su:512], kt[0:64, i * 128:(i + 1) * 128],
                qt[0:64, qlo + su:qlo + 512], start=True, stop=True)
            nc.tensor.matmul(
                psa[:, 512 + su:1024], kt[64:128, i * 128:(i + 1) * 128],
                qt[64:128, qlo + su:qlo + 512], start=True, stop=True)
            ns_pe[0] += (512 - su) / 2.4

        def emit_exp(g):
            psa, su = psas[g]
            pt = ptp.tile([128, 1024], bf16, tag="pt", name="ptile")
            pts[g] = pt
            p3i = psa[:].rearrange("p (g c) -> p g c", g=2)
            p3o = pt[:].rearrange("p (g c) -> p g c", g=2)
            nc.scalar.activation(p3o[:, :, su:512], p3i[:, :, su:512], AF.Exp)
            ns_act[0] += 2 * (512 - su) / 1.2 + 260

        def get_psy(r):
            if r not in psy_cur:
                _, jc, m, _ = STEPS[g_of[(r, 0)]]
                psyA = psY.tile([128, 512], f32, tag="psyA", name="psyAt")
                psyB = psY.tile([128, 512], f32, tag="psyB", name="psyBt")
                # accumulate onto pre-zeroed psum without HW start/stop
                # groups (start_tensor_calc's lazy zeroing wipes bank
                # neighbours), so completed slices can be read mid-round
                nc.vector.memset(psyA[:, 0:260], 0.0)
                nc.vector.memset(psyB[:, 0:260], 0.0)
                psy_cur[r] = (
                    psyA[:, 0:260].rearrange("p (j c) -> p j c", c=65),
                    psyB[:, 0:260].rearrange("p (j c) -> p j c", c=65),
                    2 * m, 2 * m + 1, jc, psyA)
            return psy_cur[r]

        def keepalive(r):
            # HAM stays at K=8/8 only with sustained PE activity; when the
            # filler pool runs dry (late jc=3 rounds) issue a dummy matmul
            # into the unused cols of the round's psy bank. start=False:
            # start_tensor_calc's lazy zeroing would wipe bank neighbours.
            psyA = psy_cur[r][5]
            nc.tensor.matmul(psyA[:, 384:512], mask[:], mask[:],
                             start=False, stop=False, skip_group_check=True)

        def norm_one(r, jj):
            psy3A, psy3B, hA, hB, jc = psy_cur[r][:5]
            j = 4 * jc + jj
            rA = rp.tile([128, 1], f32, tag="rA", name="rAt")
            rB = rp.tile([128, 1], f32, tag="rB", name="rBt")
            nc.vector.reciprocal(rA[:], psy3A[:, jj, 64:65])
            nc.vector.reciprocal(rB[:], psy3B[:, jj, 64:65])
            nc.vector.tensor_scalar_mul(
                ynall[:, j, 64 * hA:64 * hA + 64], psy3A[:, jj, 0:64], rA[:])
            nc.vector.tensor_scalar_mul(
                ynall[:, j, 64 * hB:64 * hB + 64], psy3B[:, jj, 0:64], rB[:])

        def emit_norms(r):
            # batched: one reciprocal per head covering all 4 q-subtiles
            psy3A, psy3B, hA, hB, jc = psy_cur[r][:5]
            rA = rp.tile([128, 4], f32, tag="rA", name="rAt")
            rB = rp.tile([128, 4], f32, tag="rB", name="rBt")
            rA3 = rA[:].rearrange("p (a b) -> p a b", b=1)
            rB3 = rB[:].rearrange("p (a b) -> p a b", b=1)
            nc.vector.reciprocal(rA3[:, :, :], psy3A[:, :, 64:65])
            nc.vector.reciprocal(rB3[:, :, :], psy3B[:, :, 64:65])
            for jj in range(4):
                j = 4 * jc + jj
                nc.vector.tensor_scalar_mul(
                    ynall[:, j, 64 * hA:64 * hA + 64], psy3A[:, jj, 0:64],
                    rA[:, jj:jj + 1])
                nc.vector.tensor_scalar_mul(
                    ynall[:, j, 64 * hB:64 * hB + 64], psy3B[:, jj, 0:64],
                    rB[:, jj:jj + 1])

        def emit_mask_s3(g):
            rnd_, jc, m, i = STEPS[g]
            pt = pts[g]
            _, su = psas[g]
            psy3A, psy3B, hA, hB = get_psy(rnd_)[:4]
            if 128 * i >= 512 * jc:  # diagonal block: mask keys > query
                eng = nc.gpsimd
                eng.tensor_tensor(
                    pt[:, su:su + 128], pt[:, su:su + 128], mask[:], ALU.mult)
                eng.tensor_tensor(
                    pt[:, 512 + su:512 + su + 128],
                    pt[:, 512 + su:512 + su + 128], mask[:], ALU.mult)
            jj0 = max(0, i - 4 * jc)
            # diagonal slice jj0 last: it additionally waits on the mask
            order = list(range(jj0 + 1, 4)) + [jj0] \
                if 128 * i >= 512 * jc else list(range(jj0, 4))
            for jj in order:
                nc.tensor.matmul(
                    psy3A[:, jj, :], pt[:, 128 * jj:128 * jj + 128],
                    vt[i][:, 65 * hA:65 * hA + 65],
                    start=False, stop=False, skip_group_check=True)
                nc.tensor.matmul(
                    psy3B[:, jj, :], pt[:, 512 + 128 * jj:512 + 128 * jj + 128],
                    vt[i][:, 65 * hB:65 * hB + 65],
                    start=False, stop=False, skip_group_check=True)
            ns_pe[0] += (4 - jj0) * 2 * 65 / 2.4

        # ---- main pipeline. S2/exp run one step ahead; S3 consumption lags
        # its exp by a full phase so the in-order PE queue never waits on
        # the activation latency. ----
        emit_s2(0)
        emit_exp(0)
        for ph in range(NSTEPS + 1):
            if ph + 1 < NSTEPS:
                emit_s2(ph + 1)
                emit_exp(ph + 1)
            drain_fillers(min(ph, NSTEPS - 1))
            g = ph - 1
            if g < 0:
                continue
            rnd_, jc, m, i = STEPS[g]
            emit_mask_s3(g)
            imax = 4 * jc + 4
            if rnd_ == LAST_RND and i >= 4 * jc:
                # stream the tail: per completed q-subtile, normalize,
                # transpose on the PE (no DMA round-trip), and run the
                # one-step-lagged projection
                jj = i - 4 * jc
                j = 4 * jc + jj
                norm_one(rnd_, jj)
                ptr = psA.tile([128, 512], bf16, tag="psa", name="ptrt")
                for kk in range(4):
                    nc.tensor.transpose(
                        ptr[:, 128 * kk:128 * (kk + 1)],
                        ynall[:, j, 128 * kk:128 * (kk + 1)], ident[:])
                nc.vector.tensor_copy(
                    ysbT[:, :, 128 * j:128 * (j + 1)],
                    ptr[:].rearrange("p (a b) -> p a b", b=128))
                if jj >= 1:
                    s4_part(j - 1, 0)
                    s4_part(j - 1, 1)
            elif i == imax - 1:
                emit_norms(rnd_)
                if rnd_ + 1 <= LAST_RND:
                    get_psy(rnd_ + 1)   # allocate+zero next psy behind norms
                if m == 3:  # jc-set complete: transpose its q-tiles
                    nxt = rnd_ + 1
                    for jj in range(4):
                        j = 4 * jc + jj
                        nc.sync.dma_start_transpose(
                            out=ysbT[:, :, 128 * j:128 * (j + 1)],
                            in_=ynall[:, j, :])
                        # spread the projections over the next jc-set,
                        # activating only after the transposes have landed
                        if nxt + jj <= LAST_RND:
                            dl = g_of[(nxt + jj, RLEN[nxt + jj] // 2)]
                        else:
                            dl = NSTEPS - 1
                        staged.append(
                            [g + 8 + 2 * jj, dl,
                             [lambda j=j, oc=oc: s4_part(j, oc)
                              for oc in range(2)]])

        drain_fillers(NSTEPS, force_all=True)
        # last streamed q-tile
        s4_tail(4 * JC_ORDER[-1] + 3)

    nc.compile()
    return nc


def _prep_core_inputs(x, w_qkv, w_proj, c):
    b, g = c // 2, c % 2
    scale = np.float32(D_HEAD ** -0.5)
    wq = (w_qkv[g * HD:(g + 1) * HD] * scale)
    wk = w_qkv[D_MODEL + g * HD:D_MODEL + (g + 1) * HD]
    wv = w_qkv[2 * D_MODEL + g * HD:2 * D_MODEL + (g + 1) * HD]
    tri = np.triu(np.ones((128, 128), dtype=np.float32))
    bf = ml_dtypes.bfloat16

    def chunked(a, nchunk):
        # [nchunk*128, cols] -> [128, nchunk, cols]
        return np.ascontiguousarray(
            a.reshape(nchunk, 128, a.shape[1]).transpose(1, 0, 2)).astype(bf)

    # interleave q/k column blocks per head-pair: [q0 k0 q1 k1 q2 k2 q3 k3]
    wqk_rows = []
    for m in range(4):
        wqk_rows.append(wq[128 * m:128 * (m + 1)])
        wqk_rows.append(wk[128 * m:128 * (m + 1)])
    return {
        "xT": chunked(np.ascontiguousarray(x[b].T), NTK),
        "wqkT": chunked(np.ascontiguousarray(
            np.concatenate(wqk_rows, 0).T), NTK),
        "wvT": chunked(np.ascontiguousarray(wv.T), NTK),
        "wpT": chunked(np.ascontiguousarray(
            w_proj[:, g * HD:(g + 1) * HD].T), HD // 128),
        "trimask": tri.astype(bf),
        "identT": np.eye(128, dtype=np.float32).astype(bf),
    }


def kernel(x, w_qkv, w_proj):
    x = np.asarray(x)
    w_qkv = np.asarray(w_qkv)
    w_proj = np.asarray(w_proj)
    if "nc" not in _cache:
        _cache["nc"] = _build()
    nc = _cache["nc"]
    in_maps = [_prep_core_inputs(x, w_qkv, w_proj, c) for c in range(N_CORES)]
    res = run_bass_kernel_spmd(nc, in_maps, core_ids=list(range(N_CORES)))
    outs = [res.results[c]["out"] for c in range(N_CORES)]
    return np.stack([outs[2 * b] + outs[2 * b + 1] for b in range(B)], 0)
